# revision 54
# baseline (speedup 1.0000x reference)
import math
import os
import sys

sys.path.insert(0, "/opt/trn_rl_repo")

import numpy as np
from contextlib import ExitStack

import concourse.tile as tile
from concourse import bacc, mybir
from concourse.bass_utils import run_bass_kernel_spmd

F32R = mybir.dt.float32r
FP32 = mybir.dt.float32
BF16 = mybir.dt.bfloat16
I16 = mybir.dt.int16

B, S, D, H, HD = 2, 2048, 1024, 16, 64
NCORES = 8
GH = 4            # heads per core (head group)
GW = GH * HD      # 256 columns of each projection per core
EXP = mybir.ActivationFunctionType.Exp
COPY = mybir.ActivationFunctionType.Copy
LN = mybir.ActivationFunctionType.Ln
MUL = mybir.AluOpType.mult
ADD = mybir.AluOpType.add

# Schraudolph exp in bf16: exp(x) ~= bitcast_bf16(int16(A*x + B)); applied to
# the raw (unscaled) scores, so fold the 1/sqrt(HD) softmax scale into A.
SCH_A = (128.0 / math.log(2.0)) * 0.125
SCH_B = float(127 * 128) - 5.6

_NC = None
LAST_EXEC_NS = None


def _ctx_mms(nc, cps, v_t, kt, pt_e, pt_o, p):
    for par, pt in ((0, pt_e), (1, pt_o)):
        h = 2 * p + par
        for qb in range(2):
            nc.tensor.matmul(cps[par][qb][:], v_t[:, kt, h, 0:66],
                             pt[:, 512 * qb:512 * qb + 512].bitcast(BF16),
                             start=(kt == 0), stop=(kt == 15))


def _build():
    nc = bacc.Bacc("TRN2", target_bir_lowering=False, debug=False, num_devices=1)
    xT = nc.dram_tensor("xT", [D, S], FP32, kind="ExternalInput").ap()
    wq = nc.dram_tensor("wq", [D, GW], FP32, kind="ExternalInput").ap()
    wk = nc.dram_tensor("wk", [D, GW], FP32, kind="ExternalInput").ap()
    wv = nc.dram_tensor("wv", [D, GW], FP32, kind="ExternalInput").ap()
    bq = nc.dram_tensor("bq", [GW], FP32, kind="ExternalInput").ap()
    wo = nc.dram_tensor("wo", [GW, D], FP32, kind="ExternalInput").ap()
    # transposed output: outT[d, s] in bf16 (host sums partials in fp64);
    # halves the out-DMA traffic and its end-of-kernel drain
    out = nc.dram_tensor("out", [D, S], BF16, kind="ExternalOutput").ap()

    with tile.TileContext(nc) as tc, ExitStack() as ctx:
        sb = ctx.enter_context(tc.tile_pool(name="sb", bufs=1))
        sbx = ctx.enter_context(tc.tile_pool(name="sbx", bufs=16))
        sbp = ctx.enter_context(tc.tile_pool(name="sbp", bufs=4))
        sbn = ctx.enter_context(tc.tile_pool(name="sbn", bufs=1))
        sbo = ctx.enter_context(tc.tile_pool(name="sbo", bufs=4))
        pss = ctx.enter_context(tc.tile_pool(name="pss", bufs=4, space="PSUM"))
        psc = ctx.enter_context(tc.tile_pool(name="psc", bufs=1, space="PSUM"))

        # ---- persistent weights / constants ----
        wq_t = sb.tile([128, 8, GW], F32R, name="wq_t")
        wk_t = sb.tile([128, 8, GW], F32R, name="wk_t")
        wv_t = sb.tile([128, 8, GW], F32R, name="wv_t")
        for i in range(8):
            nc.sync.dma_start(wq_t[:, i, :], wq[128 * i:128 * i + 128, :].bitcast(F32R))
            nc.sync.dma_start(wk_t[:, i, :], wk[128 * i:128 * i + 128, :].bitcast(F32R))
            nc.sync.dma_start(wv_t[:, i, :], wv[128 * i:128 * i + 128, :].bitcast(F32R))
        wo_t = sb.tile([128, 2, D], F32R, name="wo_t")
        for j in range(2):
            nc.gpsimd.dma_start(wo_t[:, j, :], wo[128 * j:128 * j + 128, :].bitcast(F32R))
        bq_t = sb.tile([128, 2], FP32, name="bq_t")
        for p in range(2):
            nc.gpsimd.dma_start(bq_t[:, p:p + 1], bq[128 * p:128 * p + 128])

        qt_t = sb.tile([128, 2, S], BF16, name="qt_t")
        kt_t = sb.tile([128, 2, S], BF16, name="kt_t")
        v_t = sb.tile([128, 16, GH, 66], BF16, name="v_t")
        cat_t = sb.tile([128, 2, S], F32R, name="cat_t")

        # ones column (64) and zero pad column (65) of the augmented V
        nc.vector.memset(v_t[:, :, :, 64:65], 1.0)
        nc.vector.memset(v_t[:, :, :, 65:66], 0.0)
        ones_b = sb.tile([1, 64], BF16, name="ones_b")
        nc.vector.memset(ones_b[:], 1.0)

        # preload the Exp activation table while the scalar engine is idle
        warm = sb.tile([1, 8], FP32, name="warm")
        nc.vector.memset(warm[:], 0.0)
        nc.scalar.activation(warm[:], warm[:], EXP)

        # ---- QKV generation ----
        # qt/kt layout: [d-in-pair(128), pair p, seq]; pair p holds heads (2p, 2p+1)
        # v layout: [k-in-block(128), kblock(16), head(4), 64 vdims + 1 + pad]
        for sbk in range(4):
            xts = []
            for i in range(8):
                xt = sbx.tile([128, 512], F32R, tag="xt")
                eng = nc.sync if i % 2 == 0 else nc.gpsimd
                eng.dma_start(xt[:], xT[128 * i:128 * i + 128,
                                        512 * sbk:512 * sbk + 512].bitcast(F32R))
                xts.append(xt)
            ssl = slice(512 * sbk, 512 * sbk + 512)
            for p in range(2):
                pq = pss.tile([128, 512], FP32, tag="sc")
                for i in range(8):
                    nc.tensor.matmul(pq[:], wq_t[:, i, 128 * p:128 * p + 128],
                                     xts[i][:], start=(i == 0), stop=(i == 7))
                nc.vector.tensor_scalar_add(qt_t[:, p, ssl], pq[:], bq_t[:, p:p + 1])
                pk = pss.tile([128, 512], FP32, tag="sc")
                for i in range(8):
                    nc.tensor.matmul(pk[:], wk_t[:, i, 128 * p:128 * p + 128],
                                     xts[i][:], start=(i == 0), stop=(i == 7))
                nc.vector.tensor_copy(kt_t[:, p, ssl], pk[:])
            for j in range(4):
                st = 4 * sbk + j
                pv = pss.tile([128, GW], FP32, tag="sc")
                for i in range(8):
                    nc.tensor.matmul(pv[:], xts[i][:, 128 * j:128 * j + 128],
                                     wv_t[:, i, :], start=(i == 0), stop=(i == 7))
                for h in range(GH):
                    nc.vector.tensor_copy(v_t[:, st, h, 0:64],
                                          pv[:, 64 * h:64 * h + 64])

        # ---- attention main loop ----
        # norm(bi) runs split: stage0 (PSUM->SBUF copies + reciprocals, no PE)
        # right after bi's last ctx matmul; the PE broadcast + the multiply into
        # cat_t are deferred into the next bi so the PE never waits on them.
        def norm_stage0(bi_state):
            cps, p, qh = bi_state["cps"], bi_state["p"], bi_state["qh"]
            css, sums = [], []
            for j, (par, qb) in enumerate([(par, qb) for par in range(2)
                                           for qb in range(2)]):
                cp = cps[par][qb]
                cs = sbn.tile([65, 512], FP32, tag=f"cs{j}", name=f"cs{j}")
                nc.scalar.activation(cs[:], cp[0:65, :], COPY)
                # bf16 copy of the sum row feeds the (bf16) broadcast matmul
                sm = sbn.tile([1, 512], BF16, tag=f"sm{j}", name=f"sm{j}")
                nc.scalar.activation(sm[:], cs[64:65, :], COPY)
                css.append(cs)
                sums.append(sm)
            bi_state["css"], bi_state["sums"] = css, sums

        def norm_finish(bi_state):
            p, qh = bi_state["p"], bi_state["qh"]
            q0 = 1024 * qh
            rps = []
            for qb in range(2):
                # pack both head-parities' sums into one [128,512] broadcast
                # (par0 -> partitions 0:64, par1 -> 64:128) so a single DVE
                # reciprocal covers both (its cost depends only on free size)
                pbq = psc.tile([128, 512], FP32, tag=f"ctx0{qb}",
                               name=f"pb{qb}")
                nc.tensor.matmul(pbq[0:64, :], ones_b[:],
                                 bi_state["sums"][qb][:],
                                 start=True, stop=True)
                nc.tensor.matmul(pbq[64:128, :], ones_b[:],
                                 bi_state["sums"][2 + qb][:],
                                 start=True, stop=True)
                rp = sbn.tile([128, 512], FP32, tag=f"rp{qb}", name=f"rp{qb}")
                nc.vector.reciprocal(rp[:], pbq[:])
                # stt requires both tensor operands to start at the same
                # partition; realign par1's half to partition 0 via DMA
                rq = sbn.tile([64, 512], FP32, tag=f"rq{qb}", name=f"rq{qb}")
                nc.sync.dma_start(rq[:], rp[64:128, :])
                rps.append((rp, rq))
            for j, (par, qb) in enumerate([(par, qb) for par in range(2)
                                           for qb in range(2)]):
                qsl = slice(q0 + 512 * qb, q0 + 512 * qb + 512)
                in1 = rps[qb][0][0:64, :] if par == 0 else rps[qb][1][:]
                nc.vector.scalar_tensor_tensor(
                    cat_t[64 * par:64 * par + 64, p, qsl],
                    bi_state["css"][j][0:64, :], 1.0, in1, MUL, MUL)

        pending = None
        for bi, (p, qh) in enumerate([(0, 0), (1, 0), (0, 1), (1, 1)]):
            q0 = 1024 * qh
            cps = None
            prevs = []
            for kt in range(16):
                pes, pos = [], []
                for qb in range(2):
                    qsl = slice(q0 + 512 * qb, q0 + 512 * qb + 512)
                    ph = pss.tile([128, 512], FP32, tag="sc", name="pe")
                    nc.tensor.matmul(ph[:],
                                     kt_t[0:64, p, 128 * kt:128 * kt + 128],
                                     qt_t[0:64, p, qsl], start=True, stop=True)
                    pes.append(ph)
                for qb in range(2):
                    qsl = slice(q0 + 512 * qb, q0 + 512 * qb + 512)
                    ph = pss.tile([128, 512], FP32, tag="sc", name="po")
                    nc.tensor.matmul(ph[:],
                                     kt_t[64:128, p, 128 * kt:128 * kt + 128],
                                     qt_t[64:128, p, qsl], start=True, stop=True)
                    pos.append(ph)
                # per-half score tiles: each half's buffer is released as
                # soon as its own activation drains it (4-deep sc rotation)
                pt_e = sbp.tile([128, 1024], BF16, tag="pt", name="pt_e")
                for qb in range(2):
                    hs = slice(512 * qb, 512 * qb + 512)
                    nc.scalar.activation(pt_e[:, hs], pes[qb][:], EXP,
                                         scale=0.125)
                pt_oi = sbp.tile([128, 1024], I16, tag="pto", name="pt_o")
                for qb in range(2):
                    hs = slice(512 * qb, 512 * qb + 512)
                    nc.vector.tensor_scalar(pt_oi[:, hs], pos[qb][:],
                                            SCH_A, SCH_B, MUL, ADD)
                if kt == 2:
                    # finish previous bi's normalization, then claim the ctx
                    # accumulator banks for this bi (tenancy order matters);
                    # the 2-deep ctx pipeline gives the reciprocal chain until
                    # kt==2 to produce rc before the pb matmuls need it
                    if pending is not None:
                        norm_finish(pending)
                        pending = None
                    cps = [[psc.tile([66, 512], FP32, tag=f"ctx{par}{qb}",
                                     name=f"ctx{par}{qb}")
                            for qb in range(2)] for par in range(2)]
                if len(prevs) == 2:
                    pr = prevs.pop(0)
                    _ctx_mms(nc, cps, v_t, pr[0], pr[1], pr[2], p)
                prevs.append((kt, pt_e, pt_oi))
            for pr in prevs:
                _ctx_mms(nc, cps, v_t, pr[0], pr[1], pr[2], p)
            pending = {"cps": cps, "p": p, "qh": qh}
            norm_stage0(pending)

        # ---- output projection: outT[d, q] = sum_c wo[c, d] * cat[c, q] ----
        # queries 0:1024 need only bi0/bi1 norms (done); emit those qblocks
        # first, then finish the last bi's norm, then the rest.
        def outproj(qbs):
            idx = 0
            for qb in qbs:
                qsl = slice(512 * qb, 512 * qb + 512)
                for dc in range(8):
                    r = idx % 8
                    if r < 4:
                        acc = psc.tile([128, 512], FP32,
                                       tag=f"ctx{r // 2}{r % 2}", name=f"po{r}")
                    else:
                        acc = pss.tile([128, 512], FP32, tag="sc", name="poa")
                    so = sbo.tile([128, 512], BF16, tag="so", name="so")
                    for p_ in range(2):
                        nc.tensor.matmul(acc[:],
                                         wo_t[:, p_, 128 * dc:128 * dc + 128],
                                         cat_t[:, p_, qsl],
                                         start=(p_ == 0), stop=(p_ == 1))
                    if idx % 2 == 0:
                        nc.vector.tensor_copy(so[:], acc[:])
                    else:
                        nc.scalar.activation(so[:], acc[:], COPY)
                    deng = nc.gpsimd if idx % 2 == 0 else nc.sync
                    deng.dma_start(out[128 * dc:128 * dc + 128, qsl], so[:])
                    idx += 1

        outproj([0, 1])
        norm_finish(pending)
        outproj([2, 3])
    nc.compile()
    return nc


def _ensure_ntff_hook():
    # bass_utils' trace path imports antenv.axon_hooks, which this image
    # lacks. Register an equivalent ctypes-based hook against the axon
    # PJRT .so (same ABI trn_agent_boot uses).
    try:
        from antenv.axon_hooks import get_axon_ntff_profile_hook  # noqa: F401
        return True
    except ImportError:
        pass
    try:
        import contextlib
        import ctypes
        import types

        import antenv

        so_path = "/opt/axon/libaxon_pjrt.so"
        lib = ctypes.CDLL(so_path)
        if not hasattr(lib, "axon_start_nrt_profile"):
            return False
        lib.axon_start_nrt_profile.argtypes = [
            ctypes.POINTER(ctypes.c_int64),
            ctypes.c_size_t,
        ]
        lib.axon_start_nrt_profile.restype = ctypes.c_int64
        lib.axon_stop_nrt_profile.argtypes = [ctypes.c_char_p]
        lib.axon_stop_nrt_profile.restype = ctypes.c_int64

        @contextlib.contextmanager
        def _hook(output_dir, device_ids):
            import jax

            jax.devices()
            if device_ids:
                ids = (ctypes.c_int64 * len(device_ids))(*device_ids)
                rc = lib.axon_start_nrt_profile(ids, len(device_ids))
            else:
                rc = lib.axon_start_nrt_profile(None, 0)
            if rc != 0:
                raise RuntimeError(f"axon_start_nrt_profile rc={rc}")
            try:
                yield
            finally:
                n = lib.axon_stop_nrt_profile(str(output_dir).encode())
                print(f"profile: {n} file(s) written to {output_dir}",
                      file=sys.stderr)

        mod = types.ModuleType("antenv.axon_hooks")
        mod.get_axon_ntff_profile_hook = lambda: _hook
        mod.set_axon_ntff_profile_hook = lambda h: None
        sys.modules["antenv.axon_hooks"] = mod
        antenv.axon_hooks = mod
        return True
    except Exception:
        return False


def kernel(**inputs):
    global _NC, LAST_EXEC_NS
    x = inputs["x"]
    wq, bq = inputs["wq"], inputs["bq"]
    wk = inputs["wk"]
    wv, bv = inputs["wv"], inputs["bv"]
    wo, bo = inputs["wo"], inputs["bo"]

    if _NC is None:
        _NC = _build()

    in_maps = []
    for c in range(NCORES):
        b, g = c // 4, c % 4
        cs_ = slice(GW * g, GW * g + GW)
        in_maps.append({
            "xT": np.ascontiguousarray(x[b].T).astype(np.float32),
            "wq": np.ascontiguousarray(wq[:, cs_]).astype(np.float32),
            "wk": np.ascontiguousarray(wk[:, cs_]).astype(np.float32),
            "wv": np.ascontiguousarray(wv[:, cs_]).astype(np.float32),
            "bq": np.ascontiguousarray(bq[cs_]).astype(np.float32),
            "wo": np.ascontiguousarray(wo[cs_, :]).astype(np.float32),
        })

    trace = bool(int(os.environ.get("KERNEL_TRACE", "0")))
    if trace:
        trace = _ensure_ntff_hook()
    res = run_bass_kernel_spmd(_NC, in_maps, list(range(NCORES)), trace=trace)
    LAST_EXEC_NS = res.exec_time_ns

    # bv and bo are handled on the host: softmax rows sum to 1, so
    # ctx = attn@(x@wv) + bv  =>  out += bv@wo + bo  (constant row)
    corr = bv.astype(np.float64) @ wo.astype(np.float64) + bo.astype(np.float64)
    acc = np.zeros((B, S, D), np.float64)
    for c in range(NCORES):
        acc[c // 4] += res.results[c]["out"].astype(np.float64).T
    acc += corr[None, None, :]
    return acc.astype(np.float32)
